# revision 11
# baseline (speedup 1.0000x reference)
"""Trainium2 kernel for the bilinear form y[b,k] = sum_ij x[b,i] x[b,j] W[i,j,k] + b[k].

Shapes: x (512, 784) f32, W (614656=784*784, 10) f32, b (10,) f32 -> y (512, 10) f32.

Strategy (8 NeuronCores):
  - Shard the j axis of W.reshape(784, 784, 10) across cores: 98 j's per core.
    Each core reads W/8 + full x (~2.9 MB in fp16); DMA ~= PE time (ridge).
  - Stage 1 (TensorE): U[b, (k,j)] = sum_i x[b,i] * W[i, j_shard, k], x^T tiles
    stationary, W shard moving, accumulating over 7 uniform 112-row i-tiles
    in PSUM (fp32).
  - Stage 2 (VectorE): multiply by x[b, j_shard] (broadcast over k) and reduce
    over j: y_part[b, k] = sum_j U[b, (k,j)] * x[b, j].
  - Host: y = sum_c y_part_c + b  (20 KB per core; no collectives needed).

Perf notes:
  - Matmul operands are fp16: PE streams 1 column/cycle (fp32 is 4x slower,
    fp32r self-loads weights every matmul), and DMA traffic halves.
    fp32 PSUM accumulation keeps the overall error ~1e-3.
  - Host pre-arranges xT/w into partition-major layouts so each DMA moves
    2-7 KB contiguous per partition (near-peak HBM rate), issued as a few
    large transfers split between the two HWDGE rings (sync + scalar).
  - i-tiles stream in 3 chunks; the matmul loop is i-outer so the PE can
    start after the first chunk (~0.5 MB) instead of the whole shard.
"""

import numpy as np

D = 784
B = 512
C = 10
NCORES = 8
JS = D // NCORES  # 98 j's per core
JK = JS * C  # 980 free columns per core, laid out as (k, j)
HALF = JK // 2  # 490 = 5 k's x 98 j's -> one PSUM bank
P = 128
B_TILES = B // P  # 4
IT = 7  # i-tiles
IP = D // IT  # 112 rows per i-tile (uniform, no padding)
N_WARMUP_MM = 12  # dummy matmuls that warm the PE clock gate during DMA wait

MM_DTYPE = "float16"  # dtype of the matmul operands (and their DMA)

_nc_cache = {}


def _build_nc():
    import concourse.bacc as bacc
    import concourse.mybir as mybir
    import concourse.tile as tile

    mm_dt = getattr(mybir.dt, MM_DTYPE)

    nc = bacc.Bacc("TRN2", target_bir_lowering=False)

    # Partition-major DRAM layouts (see _make_in_maps).
    xT = nc.dram_tensor("xT", [IP, IT, B], mm_dt, kind="ExternalInput")
    w = nc.dram_tensor("w", [2, IP, IT, HALF], mm_dt, kind="ExternalInput")
    xs = nc.dram_tensor("xs", [P, B_TILES, JS], mybir.dt.float32, kind="ExternalInput")
    y = nc.dram_tensor("y", [P, B_TILES, C], mybir.dt.float32, kind="ExternalOutput")

    with tile.TileContext(nc) as tc:
        with (
            tc.tile_pool(name="wpool", bufs=6) as wpool,
            tc.tile_pool(name="xpool", bufs=3) as xpool,
            tc.tile_pool(name="xspool", bufs=1) as xspool,
            tc.tile_pool(name="ypool", bufs=1) as ypool,
            tc.tile_pool(name="scratch", bufs=4) as spool,
            tc.tile_pool(name="psum", bufs=8, space="PSUM") as psum_pool,
        ):
            # x^T on the scalar HWDGE ring; w halves on the sync ring.
            xt_sb = xpool.tile([IP, IT, B], mm_dt, name="xt", tag="xt")
            nc.scalar.dma_start(xt_sb[:], xT[:])
            w_sb = {}
            for h in range(2):
                wt = wpool.tile([IP, IT, HALF], mm_dt, name=f"w_h{h}", tag="w")
                nc.sync.dma_start(wt[:], w[h])
                w_sb[h] = wt
            xs_sb = xspool.tile([P, B_TILES, JS], mybir.dt.float32)
            nc.scalar.dma_start(xs_sb[:], xs[:])

            # PE warmup: dummy matmuls with no DMA dependency keep the PE busy
            # while the first DMAs land, so real matmuls run at 2.4 GHz.
            dmy_s = spool.tile([IP, P], mm_dt, name="dmy_s", tag="dmy_s")
            dmy_m = spool.tile([IP, HALF], mm_dt, name="dmy_m", tag="dmy_m")
            nc.gpsimd.memset(dmy_s[:], 0.0)
            nc.gpsimd.memset(dmy_m[:], 0.0)
            dmy_p = psum_pool.tile([P, HALF], mybir.dt.float32, name="dmy_p", tag="pt")
            for _ in range(N_WARMUP_MM):
                nc.tensor.matmul(
                    dmy_p[:], dmy_s[:], dmy_m[:], start=True, stop=True
                )

            y_t = ypool.tile([P, B_TILES, C], mybir.dt.float32)
            for h in range(2):
                for bt in range(B_TILES):
                    pt = psum_pool.tile(
                        [P, HALF], mybir.dt.float32, name=f"pt_h{h}b{bt}", tag="pt"
                    )
                    for it in range(IT):
                        nc.tensor.matmul(
                            pt[:],
                            xt_sb[:, it, bt * P : (bt + 1) * P],
                            w_sb[h][:, it, :],
                            start=(it == 0),
                            stop=(it == IT - 1),
                        )
                    scr = spool.tile([P, HALF], mybir.dt.float32)
                    pt3 = pt[:].rearrange("p (k j) -> p k j", k=C // 2)
                    scr3 = scr[:].rearrange("p (k j) -> p k j", k=C // 2)
                    xs3 = xs_sb[:, bt, None, :].broadcast_to([P, C // 2, JS])
                    nc.vector.tensor_tensor(scr3, pt3, xs3, mybir.AluOpType.mult)
                    nc.vector.tensor_reduce(
                        out=y_t[:, bt, h * 5 : (h + 1) * 5],
                        in_=scr3,
                        op=mybir.AluOpType.add,
                        axis=mybir.AxisListType.X,
                    )
            nc.scalar.dma_start(y[:], y_t[:])

    nc.compile()
    return nc


def _get_nc():
    if "nc" not in _nc_cache:
        _nc_cache["nc"] = _build_nc()
    return _nc_cache["nc"]


def _make_in_maps(x, W):
    import concourse.mybir as mybir

    mm_np = mybir.dt.np(getattr(mybir.dt, MM_DTYPE))
    x = np.asarray(x, dtype=np.float32)
    Wr = np.asarray(W, dtype=np.float32).reshape(D, D, C)
    # xT_dram[p, t, b] = x[b, t*IP + p]
    xT = np.ascontiguousarray(
        x.T.astype(mm_np).reshape(IT, IP, B).transpose(1, 0, 2)
    )
    # xs_dram[p, t, j] = x[t*P + p, js + j]  (per-core slice below)
    in_maps = []
    for c in range(NCORES):
        js, je = c * JS, (c + 1) * JS
        # wsh[i, k*JS + j] = W[i, js+j, k]; then [h, p, t, col] partition-major
        wsh = Wr[:, js:je, :].transpose(0, 2, 1).reshape(D, JK).astype(mm_np)
        wshard = np.ascontiguousarray(
            wsh.reshape(IT, IP, 2, HALF).transpose(2, 1, 0, 3)
        )
        xsl = np.ascontiguousarray(
            x[:, js:je].reshape(B_TILES, P, JS).transpose(1, 0, 2)
        )
        in_maps.append({"xT": xT, "w": wshard, "xs": xsl})
    return in_maps


def run_spmd(x, W, **spmd_kwargs):
    """Compile/run the SPMD kernel; returns (partials, BassKernelResults)."""
    from concourse.bass_utils import run_bass_kernel_spmd

    nc = _get_nc()
    in_maps = _make_in_maps(x, W)
    res = run_bass_kernel_spmd(nc, in_maps, core_ids=list(range(NCORES)), **spmd_kwargs)
    # y_dram[p, t, k] -> y[t*P + p, k]
    partials = [
        r["y"].transpose(1, 0, 2).reshape(B, C) for r in res.results
    ]
    return partials, res


def kernel(x, W, b):
    partials, _ = run_spmd(x, W)
    y = np.sum(np.stack(partials, 0), axis=0, dtype=np.float64) + np.asarray(
        b, dtype=np.float64
    )
    return y.astype(np.float32)


# revision 15
# speedup vs baseline: 1.1833x; 1.1833x over previous
"""Trainium2 kernel for the bilinear form y[b,k] = sum_ij x[b,i] x[b,j] W[i,j,k] + b[k].

Shapes: x (512, 784) f32, W (614656=784*784, 10) f32, b (10,) f32 -> y (512, 10) f32.

Strategy (8 NeuronCores):
  - Shard the j axis of W.reshape(784, 784, 10) across cores: 98 j's per core.
    Each core reads W/8 + full x (~2.9 MB in fp16); DMA ~= PE time (ridge).
  - Stage 1 (TensorE): U[b, (k,j)] = sum_i x[b,i] * W[i, j_shard, k], x^T tiles
    stationary, W shard moving, accumulating over 7 uniform 112-row i-tiles
    in PSUM (fp32).
  - Stage 2 (VectorE): multiply by x[b, j_shard] (broadcast over k) and reduce
    over j: y_part[b, k] = sum_j U[b, (k,j)] * x[b, j].
  - Host: y = sum_c y_part_c + b  (20 KB per core; no collectives needed).

Perf notes:
  - Matmul operands are fp16: PE streams 1 column/cycle (fp32 is 4x slower,
    fp32r self-loads weights every matmul), and DMA traffic halves.
    fp32 PSUM accumulation keeps the overall error ~1e-3.
  - Host pre-arranges xT/w into partition-major layouts so each DMA moves
    2-7 KB contiguous per partition (near-peak HBM rate), issued as a few
    large transfers split between the two HWDGE rings (sync + scalar).
  - i-tiles stream in 3 chunks; the matmul loop is i-outer so the PE can
    start after the first chunk (~0.5 MB) instead of the whole shard.
"""

import numpy as np

D = 784
B = 512
C = 10
NCORES = 8
JS = D // NCORES  # 98 j's per core
JK = JS * C  # 980 free columns per core, laid out as (k, j)
HALF = JK // 2  # 490 = 5 k's x 98 j's -> one PSUM bank
P = 128
B_TILES = B // P  # 4
IT = 7  # i-tiles
IP = D // IT  # 112 rows per i-tile (uniform, no padding)
N_WARMUP_MM = 24  # dummy matmuls that warm the PE clock gate during DMA wait
WARMUP_N = 128  # free dim of each warmup matmul (~107 ns each, cold)
CHUNKS = [(0, 1), (1, 3), (3, 5), (5, 7)]  # i-tile DMA chunks
PREFIX_ITS = 3  # i-tiles issued chunk-major; the rest run group-contiguous

MM_DTYPE = "float16"  # dtype of the matmul operands (and their DMA)

_nc_cache = {}


def _build_nc():
    import concourse.bacc as bacc
    import concourse.mybir as mybir
    import concourse.tile as tile

    mm_dt = getattr(mybir.dt, MM_DTYPE)

    nc = bacc.Bacc("TRN2", target_bir_lowering=False)

    # Partition-major DRAM layouts (see _make_in_maps).
    xT = nc.dram_tensor("xT", [IP, IT, B], mm_dt, kind="ExternalInput")
    w = nc.dram_tensor("w", [2, IP, IT, HALF], mm_dt, kind="ExternalInput")
    xs = nc.dram_tensor("xs", [P, B_TILES, JS], mybir.dt.float32, kind="ExternalInput")
    y = nc.dram_tensor("y", [P, B_TILES, C], mybir.dt.float32, kind="ExternalOutput")

    with tile.TileContext(nc) as tc:
        with (
            tc.tile_pool(name="wpool", bufs=8) as wpool,
            tc.tile_pool(name="xpool", bufs=4) as xpool,
            tc.tile_pool(name="xspool", bufs=1) as xspool,
            tc.tile_pool(name="ypool", bufs=1) as ypool,
            tc.tile_pool(name="scratch", bufs=4) as spool,
            tc.tile_pool(name="psum", bufs=8, space="PSUM") as psum_pool,
        ):
            # x^T chunks on the scalar HWDGE ring; w chunks on the sync ring.
            xT_sb = {}
            for c0, c1 in CHUNKS:
                xt = xpool.tile([IP, c1 - c0, B], mm_dt, name=f"xt_c{c0}", tag="xt")
                nc.scalar.dma_start(xt[:], xT[:, c0:c1, :])
                for it in range(c0, c1):
                    xT_sb[it] = xt[:, it - c0, :]
            w_sb = {}
            for h in range(2):
                for c0, c1 in CHUNKS:
                    wt = wpool.tile(
                        [IP, c1 - c0, HALF], mm_dt, name=f"w_h{h}c{c0}", tag="w"
                    )
                    nc.sync.dma_start(wt[:], w[h, :, c0:c1, :])
                    for it in range(c0, c1):
                        w_sb[(it, h)] = wt[:, it - c0, :]
            xs_sb = xspool.tile([P, B_TILES, JS], mybir.dt.float32)
            nc.scalar.dma_start(xs_sb[:], xs[:])

            # PE warmup: dummy matmuls with no DMA dependency keep the PE busy
            # while the first DMAs land, so real matmuls run at 2.4 GHz.
            dmy_s = spool.tile([IP, P], mm_dt, name="dmy_s", tag="dmy_s")
            dmy_m = spool.tile([IP, WARMUP_N], mm_dt, name="dmy_m", tag="dmy_m")
            nc.gpsimd.memset(dmy_s[:], 0.0)
            nc.gpsimd.memset(dmy_m[:], 0.0)
            dmy_p = psum_pool.tile(
                [P, WARMUP_N], mybir.dt.float32, name="dmy_p", tag="dp", bufs=1
            )
            for _ in range(N_WARMUP_MM):
                nc.tensor.matmul(
                    dmy_p[:], dmy_s[:], dmy_m[:], start=True, stop=True
                )

            y_t = ypool.tile([P, B_TILES, C], mybir.dt.float32)
            for h in range(2):
                pts = {}
                for bt in range(B_TILES):
                    pts[bt] = psum_pool.tile(
                        [P, HALF],
                        mybir.dt.float32,
                        name=f"pt_h{h}b{bt}",
                        tag="pt",
                        bufs=7,
                    )
                # Prefix i-tiles chunk-major so the PE chases DMA arrivals...
                for it in range(PREFIX_ITS):
                    for bt in range(B_TILES):
                        nc.tensor.matmul(
                            pts[bt][:],
                            xT_sb[it][:, bt * P : (bt + 1) * P],
                            w_sb[(it, h)][:],
                            start=(it == 0),
                            stop=False,
                        )
                # ...then group-contiguous so the stop matmuls stagger and the
                # vector engine's stage 2 overlaps the remaining matmuls.
                for bt in range(B_TILES):
                    pt = pts[bt]
                    for it in range(PREFIX_ITS, IT):
                        nc.tensor.matmul(
                            pt[:],
                            xT_sb[it][:, bt * P : (bt + 1) * P],
                            w_sb[(it, h)][:],
                            start=False,
                            stop=(it == IT - 1),
                        )
                    scr = spool.tile([P, HALF], mybir.dt.float32)
                    pt3 = pt[:].rearrange("p (k j) -> p k j", k=C // 2)
                    scr3 = scr[:].rearrange("p (k j) -> p k j", k=C // 2)
                    xs3 = xs_sb[:, bt, None, :].broadcast_to([P, C // 2, JS])
                    nc.vector.tensor_tensor(scr3, pt3, xs3, mybir.AluOpType.mult)
                    nc.vector.tensor_reduce(
                        out=y_t[:, bt, h * 5 : (h + 1) * 5],
                        in_=scr3,
                        op=mybir.AluOpType.add,
                        axis=mybir.AxisListType.X,
                    )
            nc.scalar.dma_start(y[:], y_t[:])

    nc.compile()
    return nc


def _get_nc():
    if "nc" not in _nc_cache:
        _nc_cache["nc"] = _build_nc()
    return _nc_cache["nc"]


def _make_in_maps(x, W):
    import concourse.mybir as mybir

    mm_np = mybir.dt.np(getattr(mybir.dt, MM_DTYPE))
    x = np.asarray(x, dtype=np.float32)
    Wr = np.asarray(W, dtype=np.float32).reshape(D, D, C)
    # xT_dram[p, t, b] = x[b, t*IP + p]
    xT = np.ascontiguousarray(
        x.T.astype(mm_np).reshape(IT, IP, B).transpose(1, 0, 2)
    )
    # xs_dram[p, t, j] = x[t*P + p, js + j]  (per-core slice below)
    in_maps = []
    for c in range(NCORES):
        js, je = c * JS, (c + 1) * JS
        # wsh[i, k*JS + j] = W[i, js+j, k]; then [h, p, t, col] partition-major
        wsh = Wr[:, js:je, :].transpose(0, 2, 1).reshape(D, JK).astype(mm_np)
        wshard = np.ascontiguousarray(
            wsh.reshape(IT, IP, 2, HALF).transpose(2, 1, 0, 3)
        )
        xsl = np.ascontiguousarray(
            x[:, js:je].reshape(B_TILES, P, JS).transpose(1, 0, 2)
        )
        in_maps.append({"xT": xT, "w": wshard, "xs": xsl})
    return in_maps


def run_spmd(x, W, **spmd_kwargs):
    """Compile/run the SPMD kernel; returns (partials, BassKernelResults)."""
    from concourse.bass_utils import run_bass_kernel_spmd

    nc = _get_nc()
    in_maps = _make_in_maps(x, W)
    res = run_bass_kernel_spmd(nc, in_maps, core_ids=list(range(NCORES)), **spmd_kwargs)
    # y_dram[p, t, k] -> y[t*P + p, k]
    partials = [
        r["y"].transpose(1, 0, 2).reshape(B, C) for r in res.results
    ]
    return partials, res


def kernel(x, W, b):
    partials, _ = run_spmd(x, W)
    y = np.sum(np.stack(partials, 0), axis=0, dtype=np.float64) + np.asarray(
        b, dtype=np.float64
    )
    return y.astype(np.float32)
